# revision 19
# baseline (speedup 1.0000x reference)
# Trainium2 Bass kernel for nn_CausalTransformer (B=64, V=256, D=512, DFF=2048, L=8, H=8).
# Data-parallel over batch: 8 rows per core x 8 cores, params replicated (no collectives).
# Activations live feature-major in SBUF ([feature-partition, token-free]); matmuls in
# bf16 with f32 PSUM accumulation; weights are pre-transposed + bf16-cast on host.
import numpy as np
import ml_dtypes

import concourse.bass as bass
import concourse.mybir as mybir
import concourse.tile as tile
from concourse import bacc
from concourse.bass_utils import run_bass_kernel_spmd
from concourse.masks import make_identity

P = 128
B, V, D, DFF, L, H = 64, 256, 512, 2048, 8, 8
HD = D // H            # 64
NCORES = 8
NROW = B // NCORES     # 8 batch rows per core
TOK = NROW * V         # 2048 tokens per core
DT = D // P            # 4 feature tiles
FT = DFF // P          # 16 ffn tiles
CW = 512               # token chunk width
NCH = TOK // CW        # 4 chunks
EPS = 1e-5
F32 = mybir.dt.float32
BF16 = mybir.dt.bfloat16
AF = mybir.ActivationFunctionType
ALU = mybir.AluOpType

LAST_RESULT = None  # BassKernelResults of the most recent run (for profiling)


def _bf(a):
    return np.ascontiguousarray(a.astype(ml_dtypes.bfloat16))


def _f32(a):
    return np.ascontiguousarray(a.astype(np.float32))


def _fm_tiles(w):
    # [Dout, Din] weight -> lhsT tiles [P, Din//P, Dout] (partition = Din within tile)
    dout, din = w.shape
    return np.ascontiguousarray(w.T.reshape(din // P, P, dout).transpose(1, 0, 2))


def _pcol(v):
    # [N] vector -> [P, N//P] column-per-tile layout (partition-major within tile)
    n = v.shape[0]
    return np.ascontiguousarray(v.reshape(n // P, P).T)


def _prep_params(params):
    p = {k: np.asarray(v) for k, v in params.items()}
    ins = {}
    ins["wqkvT"] = _bf(np.stack([_fm_tiles(p["Wqkv"][l]) for l in range(L)]))
    ins["bqkv"] = _f32(np.stack([_pcol(p["bqkv"][l]) for l in range(L)]))
    ins["woT"] = _bf(np.stack([_fm_tiles(p["Wo"][l]) for l in range(L)]))
    ins["w1T"] = _bf(np.stack([_fm_tiles(p["W1"][l]) for l in range(L)]))
    ins["b1"] = _f32(np.stack([_pcol(p["b1"][l]) for l in range(L)]))
    ins["w2T"] = _bf(np.stack([_fm_tiles(p["W2"][l]) for l in range(L)]))
    ins["ln1g"] = _f32(np.stack([_pcol(p["ln1_g"][l]) for l in range(L)]))
    ins["ln1b"] = _f32(np.stack([_pcol(p["ln1_b"][l]) for l in range(L)]))
    ins["ln2g"] = _f32(np.stack([_pcol(p["ln2_g"][l]) for l in range(L)]))
    ins["ln2b"] = _f32(np.stack([_pcol(p["ln2_b"][l]) for l in range(L)]))
    for nm in ("fe", "ve"):
        ins[nm + "w2T"] = _bf(_fm_tiles(p[nm + "_w2"]))
    sm = np.concatenate([
        _pcol(p["fe_w1"][:, 0]), _pcol(p["fe_b1"]),
        _pcol(p["ve_w1"][:, 0]), _pcol(p["ve_b1"]),
        _pcol(p["fe_g"]), _pcol(p["fe_bn"]), _pcol(p["ve_g"]), _pcol(p["ve_bn"]),
        _pcol(p["fe_b2"]), _pcol(p["ve_b2"]),
        _pcol(p["me_w"][:, 0]), _pcol(p["me_b"]),
    ], axis=1)                                   # [P, 64]
    vet = p["var_emb"].T.reshape(DT, P, V).transpose(1, 0, 2).reshape(P, DT * V)
    ins["_embtail"] = _f32(np.concatenate([sm, vet], axis=1))   # [P, 64+1024]
    ins["owT"] = _bf(_pcol(p["out_w"][0]))
    out_b = float(np.asarray(p["out_b"]).reshape(-1)[0])
    return ins, out_b


def _build(out_b_val):
    nc = bacc.Bacc()
    # ---- DRAM I/O ----
    d_emb = nc.dram_tensor("embal", [P, 3 * TOK + 64 + DT * V], F32,
                           kind="ExternalInput")
    shp = {
        "wqkvT": ([L, P, DT, 3 * D], BF16), "bqkv": ([L, P, 12], F32),
        "woT": ([L, P, DT, D], BF16),
        "w1T": ([L, P, DT, DFF], BF16), "b1": ([L, P, FT], F32),
        "w2T": ([L, P, FT, D], BF16),
        "ln1g": ([L, P, DT], F32), "ln1b": ([L, P, DT], F32),
        "ln2g": ([L, P, DT], F32), "ln2b": ([L, P, DT], F32),
        "few2T": ([P, 8, D], BF16),
        "vew2T": ([P, 8, D], BF16),
        "owT": ([P, DT], BF16),
    }
    dps = {k: nc.dram_tensor(k, s, dt, kind="ExternalInput") for k, (s, dt) in shp.items()}
    d_pred = nc.dram_tensor("pred", [1, TOK], F32, kind="ExternalOutput")
    d_avg = nc.dram_tensor("avg", [TOK, V], F32, kind="ExternalOutput")

    with tile.TileContext(nc) as tc:
        with (
            tc.tile_pool(name="state", bufs=1) as st,
            tc.tile_pool(name="lnp", bufs=2) as lnp,
            tc.tile_pool(name="linp", bufs=2) as linp,
            tc.tile_pool(name="smallp", bufs=4) as smallp,
            tc.tile_pool(name="psA", bufs=4, space="PSUM") as psA,
            tc.tile_pool(name="psS", bufs=2, space="PSUM") as psS,
            tc.tile_pool(name="psV", bufs=1, space="PSUM") as psV,
            tc.tile_pool(name="psT", bufs=1, space="PSUM") as psT,
        ):
            # ---- persistent state ----
            h_a = st.tile([P, DT, TOK], BF16, tag="h_a")
            h_b = st.tile([P, DT, TOK], BF16, tag="h_b")
            acc = st.tile([P, 2 * NROW, V], F32, tag="acc")  # [q%128, 2*r+qt, k]
            ident = st.tile([P, P], BF16, tag="ident")
            ones_bf = st.tile([P, 1], BF16, tag="ones_bf")
            ones_f2 = st.tile([P, 1], F32, tag="ones_f2")
            erow = st.tile([P, P], F32, tag="erow")
            owt = st.tile([P, DT], BF16, tag="owt")

            make_identity(nc, ident[:])
            nc.vector.memset(ones_bf[:], 1.0)
            nc.vector.memset(ones_f2[:], 1.0)
            nc.vector.memset(erow[:], 0.0)
            nc.vector.memset(erow[0:1, :], 1.0)
            nc.vector.memset(acc[:], 0.0)
            nc.sync.dma_start(owt[:], dps["owT"][:])

            def ln_chunk(src, g_ap, b_ap, out_ap, add_into=None):
                # LayerNorm over D of feature-major chunk src [P, DT, CW] (bf16).
                sq = lnp.tile([P, DT, CW], BF16, tag="lnsq")
                nc.scalar.activation(sq[:], src, AF.Square)
                s1 = psA.tile([1, CW], F32, tag="big")
                s2 = psA.tile([1, CW], F32, tag="big")
                for kt in range(DT):
                    nc.tensor.matmul(s1[:], ones_f2[:], src[:, kt, :],
                                     start=(kt == 0), stop=(kt == DT - 1))
                for kt in range(DT):
                    nc.tensor.matmul(s2[:], ones_bf[:], sq[:, kt, :],
                                     start=(kt == 0), stop=(kt == DT - 1))
                slin = linp.tile([1, 2, CW], F32, tag="lin")
                nc.scalar.copy(slin[:, 0, :], s1[:])
                nc.scalar.copy(slin[:, 1, :], s2[:])
                sst = smallp.tile([32, 2, 16], F32, tag="sst")
                for j in range(2):
                    nc.sync.dma_start(
                        sst[:, j : j + 1, :],
                        slin[:, j, :].rearrange("o (p f) -> o p f", p=32),
                    )
                mean = smallp.tile([32, 16], F32, tag="lnm")
                e2 = smallp.tile([32, 16], F32, tag="lne2")
                var = smallp.tile([32, 16], F32, tag="lnvar")
                inv = smallp.tile([32, 16], F32, tag="lninv")
                minv = smallp.tile([32, 16], F32, tag="lnminv")
                nc.vector.tensor_scalar_mul(mean[:], sst[:, 0, :], 1.0 / D)
                nc.vector.tensor_scalar_mul(e2[:], sst[:, 1, :], 1.0 / D)
                nc.vector.tensor_tensor(var[:], mean[:], mean[:], ALU.mult)
                nc.vector.tensor_tensor(var[:], e2[:], var[:], ALU.subtract)
                nc.vector.tensor_scalar_add(var[:], var[:], EPS)
                nc.scalar.activation(var[:], var[:], AF.Sqrt)
                nc.vector.reciprocal(inv[:], var[:])
                nc.vector.tensor_tensor(minv[:], mean[:], inv[:], ALU.mult)
                vecs = linp.tile([P, 2, CW], F32, tag="vecs")
                nc.vector.memset(vecs[:], 0.0)
                nc.sync.dma_start(
                    vecs[0:1, 0, :].rearrange("o (p f) -> o p f", p=32), inv[:, None, :]
                )
                nc.sync.dma_start(
                    vecs[0:1, 1, :].rearrange("o (p f) -> o p f", p=32), minv[:, None, :]
                )
                invb = psA.tile([P, CW], F32, tag="big")
                minvb = psA.tile([P, CW], F32, tag="big")
                nc.tensor.matmul(invb[:], erow[:], vecs[:, 0, :])
                nc.tensor.matmul(minvb[:], erow[:], vecs[:, 1, :])
                for kt in range(DT):
                    t = lnp.tile([P, CW], F32, tag="lnt")
                    nc.vector.tensor_tensor(t[:], src[:, kt, :], invb[:], ALU.mult)
                    nc.vector.tensor_tensor(t[:], t[:], minvb[:], ALU.subtract)
                    if add_into is None:
                        nc.vector.tensor_scalar(
                            out_ap[:, kt, :], t[:], g_ap[:, kt : kt + 1],
                            b_ap[:, kt : kt + 1], ALU.mult, ALU.add,
                        )
                    else:
                        nc.vector.tensor_scalar(
                            t[:], t[:], g_ap[:, kt : kt + 1],
                            b_ap[:, kt : kt + 1], ALU.mult, ALU.add,
                        )
                        nc.vector.tensor_tensor(
                            add_into[:, kt, :], add_into[:, kt, :], t[:], ALU.add
                        )

            # ================= Embeddings (phase-scoped pools) =================
            with (
                tc.tile_pool(name="embw", bufs=1) as ew,
                tc.tile_pool(name="embp", bufs=2) as ep,
            ):
                embal = ew.tile([P, 3 * TOK + 64 + DT * V], F32, tag="embal")
                fw2 = ew.tile([P, 8, D], BF16, tag="few2T")
                vw2 = ew.tile([P, 8, D], BF16, tag="vew2T")
                nc.sync.dma_start(embal[:], d_emb[:])
                nc.sync.dma_start(fw2[:], dps["few2T"][:])
                nc.sync.dma_start(vw2[:], dps["vew2T"][:])
                xbt = embal[:, 0:TOK]
                vbt = embal[:, TOK : 2 * TOK]
                mbt = embal[:, 2 * TOK : 3 * TOK]
                s0 = 3 * TOK
                emb = {
                    "few1": embal[:, s0 : s0 + 8], "feb1": embal[:, s0 + 8 : s0 + 16],
                    "vew1": embal[:, s0 + 16 : s0 + 24],
                    "veb1": embal[:, s0 + 24 : s0 + 32],
                    "feg": embal[:, s0 + 32 : s0 + 36],
                    "febn": embal[:, s0 + 36 : s0 + 40],
                    "veg": embal[:, s0 + 40 : s0 + 44],
                    "vebn": embal[:, s0 + 44 : s0 + 48],
                    "feb2": embal[:, s0 + 48 : s0 + 52],
                    "veb2": embal[:, s0 + 52 : s0 + 56],
                    "mew": embal[:, s0 + 56 : s0 + 60],
                    "meb": embal[:, s0 + 60 : s0 + 64],
                    "few2T": fw2, "vew2T": vw2,
                    "varembT": embal[:, s0 + 64 :].rearrange(
                        "p (kt v) -> p kt v", v=V),
                }
                for g in range(NCH):
                    gsl = bass.ds(g * CW, CW)
                    xb = xbt[:, gsl]
                    vb = vbt[:, gsl]
                    mb = mbt[:, gsl]
                    for nm, srcb in (("fe", xb), ("ve", vb)):
                        h1 = ep.tile([P, 8, CW], BF16, tag="embh1")
                        for m in range(8):
                            nc.scalar.activation(
                                h1[:, m, :], srcb, AF.Gelu,
                                scale=emb[nm + "w1"][:, m : m + 1],
                                bias=emb[nm + "b1"][:, m : m + 1],
                            )
                        esb = ep.tile([P, DT, CW], F32, tag="esb")
                        for m in range(DT):
                            ps = psA.tile([P, CW], F32, tag="big")
                            for kt in range(8):
                                nc.tensor.matmul(
                                    ps[:], emb[nm + "w2T"][:, kt, bass.ds(m * P, P)],
                                    h1[:, kt, :], start=(kt == 0), stop=(kt == 7),
                                )
                            nc.vector.tensor_scalar_add(
                                esb[:, m, :], ps[:], emb[nm + "b2"][:, m : m + 1]
                            )
                        if nm == "fe":
                            ln_chunk(esb[:], emb["feg"], emb["febn"], h_a[:, :, gsl])
                        else:
                            ln_chunk(esb[:], emb["veg"], emb["vebn"], None,
                                     add_into=h_a[:, :, gsl])
                    for kt in range(DT):
                        met = ep.tile([P, CW], F32, tag="met")
                        nc.vector.tensor_scalar(
                            met[:], mb, emb["mew"][:, kt : kt + 1],
                            emb["meb"][:, kt : kt + 1], ALU.mult, ALU.add,
                        )
                        nc.vector.tensor_tensor(
                            h_a[:, kt, gsl], h_a[:, kt, gsl], met[:], ALU.add
                        )
                        nc.vector.tensor_tensor(
                            h_a[:, kt, gsl].rearrange("p (r k) -> p r k", k=V),
                            h_a[:, kt, gsl].rearrange("p (r k) -> p r k", k=V),
                            emb["varembT"][:, kt, None, :].to_broadcast((P, CW // V, V)),
                            ALU.add,
                        )

            # ================= Transformer layers =================
            with (
                tc.tile_pool(name="wp", bufs=1) as wp,
                tc.tile_pool(name="qkvp", bufs=1) as qkvp,
                tc.tile_pool(name="attp", bufs=2) as attp,
                tc.tile_pool(name="att1", bufs=1) as att1,
                tc.tile_pool(name="midp", bufs=2) as midp,
                tc.tile_pool(name="mid1", bufs=1) as mid1,
            ):
                for l in range(L):
                    hin = h_a if l % 2 == 0 else h_b
                    hout = h_b if l % 2 == 0 else h_a
                    wqkv = wp.tile([P, DT, 3 * D], BF16, tag="wqkv")
                    bqkv = wp.tile([P, 12], F32, tag="bqkv")
                    wo = wp.tile([P, DT, D], BF16, tag="wo")
                    w1 = wp.tile([P, DT, DFF], BF16, tag="w1")
                    b1 = wp.tile([P, FT], F32, tag="b1")
                    w2 = wp.tile([P, FT, D], BF16, tag="w2")
                    lng = wp.tile([P, 4, DT], F32, tag="lng")
                    nc.sync.dma_start(wqkv[:], dps["wqkvT"][l])
                    nc.sync.dma_start(bqkv[:], dps["bqkv"][l])
                    nc.sync.dma_start(wo[:], dps["woT"][l])
                    nc.sync.dma_start(w1[:], dps["w1T"][l])
                    nc.sync.dma_start(b1[:], dps["b1"][l])
                    nc.sync.dma_start(w2[:], dps["w2T"][l])
                    for j, k in enumerate(("ln1g", "ln1b", "ln2g", "ln2b")):
                        nc.sync.dma_start(lng[:, j, :], dps[k][l])

                    for g in range(NCH):
                        gsl = bass.ds(g * CW, CW)
                        # ---- QKV projection ----
                        qkv = qkvp.tile([P, 12, CW], BF16, tag="qkv")
                        for m in range(12):
                            ps = psA.tile([P, CW], F32, tag="big")
                            for kt in range(DT):
                                nc.tensor.matmul(
                                    ps[:], wqkv[:, kt, bass.ds(m * P, P)],
                                    hin[:, kt, gsl],
                                    start=(kt == 0), stop=(kt == DT - 1),
                                )
                            nc.vector.tensor_scalar_add(
                                qkv[:, m, :], ps[:], bqkv[:, m : m + 1]
                            )
                        # ---- attention (2 rows in this chunk) ----
                        attn = attp.tile([P, DT, CW], BF16, tag="attn")
                        for rr in range(2):
                            r = 2 * g + rr
                            rsl = bass.ds(rr * V, V)
                            psb = att1.tile([P, H, 2, V], BF16, tag="psb")
                            sums = smallp.tile([P, 2, H], F32, tag="sums")
                            for hh in range(H):
                                hp, hq = (hh % 2) * 64, hh // 2
                                for qt in range(2):
                                    scps = psS.tile([P, V], F32, tag="sc")
                                    nc.tensor.matmul(
                                        scps[:],
                                        qkv[hp : hp + 64, hq,
                                            bass.ds(rr * V + qt * P, P)],
                                        qkv[hp : hp + 64, 4 + hq, rsl],
                                    )
                                    nc.scalar.activation(
                                        psb[:, hh, qt, :], scps[:], AF.Exp,
                                        scale=0.125,
                                        accum_out=sums[:, qt, hh : hh + 1],
                                    )
                            rinv = smallp.tile([P, 2, H], F32, tag="rinv")
                            nc.vector.reciprocal(rinv[:], sums[:])
                            for hh in range(H):
                                hp, hq = (hh % 2) * 64, hh // 2
                                ptT = attp.tile([P, 2, V], BF16, tag="ptT")
                                for qt in range(2):
                                    pn = attp.tile([P, V], BF16, tag="pn")
                                    nc.vector.tensor_scalar_mul(
                                        pn[:], psb[:, hh, qt, :],
                                        rinv[:, qt, hh : hh + 1],
                                    )
                                    nc.vector.tensor_tensor(
                                        acc[:, 2 * r + qt, :], acc[:, 2 * r + qt, :],
                                        pn[:], ALU.add,
                                    )
                                    tp = psT.tile([P, 2, P], BF16, tag="tr")
                                    for kt in range(2):
                                        nc.tensor.transpose(
                                            tp[:, kt, :], pn[:, bass.ds(kt * P, P)],
                                            ident[:],
                                        )
                                    nc.vector.tensor_copy(
                                        ptT[:, :, bass.ds(qt * P, P)], tp[:]
                                    )
                                vtok = attp.tile([P, 2, HD], BF16, tag="vtok")
                                tv = psT.tile([P, 2, P], BF16, tag="tr", name="tv")[:, :, :HD]
                                for kt in range(2):
                                    nc.tensor.transpose(
                                        tv[:, kt, :],
                                        qkv[hp : hp + 64, 8 + hq,
                                            bass.ds(rr * V + kt * P, P)],
                                        ident[hp : hp + 64, bass.ds(hp, 64)],
                                    )
                                nc.vector.tensor_copy(vtok[:], tv[:])
                                avps = psV.tile([64, V], F32, tag="av")
                                for qt in range(2):
                                    for kt in range(2):
                                        nc.tensor.matmul(
                                            avps[:, bass.ds(qt * P, P)],
                                            vtok[:, kt, :],
                                            ptT[:, kt, bass.ds(qt * P, P)],
                                            start=(kt == 0), stop=(kt == 1),
                                        )
                                nc.scalar.copy(attn[hp : hp + 64, hq, rsl], avps[:])
                        # ---- Wo + residual ----
                        hmid = mid1.tile([P, DT, CW], F32, tag="hmid")
                        for m in range(DT):
                            ps = psA.tile([P, CW], F32, tag="big")
                            for kt in range(DT):
                                nc.tensor.matmul(
                                    ps[:], wo[:, kt, bass.ds(m * P, P)],
                                    attn[:, kt, :],
                                    start=(kt == 0), stop=(kt == DT - 1),
                                )
                            nc.vector.tensor_tensor(
                                hmid[:, m, :], ps[:], hin[:, m, gsl], ALU.add
                            )
                        # ---- LN1 ----
                        hln = midp.tile([P, DT, CW], BF16, tag="hln")
                        ln_chunk(hmid[:], lng[:, 0, :], lng[:, 1, :], hln)
                        # ---- FFN ----
                        mid = mid1.tile([P, FT, CW], BF16, tag="mid")
                        for m in range(FT):
                            ps = psA.tile([P, CW], F32, tag="big")
                            for kt in range(DT):
                                nc.tensor.matmul(
                                    ps[:], w1[:, kt, bass.ds(m * P, P)],
                                    hln[:, kt, :],
                                    start=(kt == 0), stop=(kt == DT - 1),
                                )
                            nc.scalar.activation(
                                mid[:, m, :], ps[:], AF.Relu, bias=b1[:, m : m + 1]
                            )
                        h2m = mid1.tile([P, DT, CW], F32, tag="h2m")
                        for m in range(DT):
                            ps = psA.tile([P, CW], F32, tag="big")
                            for kt in range(FT):
                                nc.tensor.matmul(
                                    ps[:], w2[:, kt, bass.ds(m * P, P)],
                                    mid[:, kt, :],
                                    start=(kt == 0), stop=(kt == FT - 1),
                                )
                            nc.vector.tensor_tensor(
                                h2m[:, m, :], ps[:], hln[:, m, :], ALU.add
                            )
                        # ---- LN2 ----
                        ln_chunk(h2m[:], lng[:, 2, :], lng[:, 3, :], hout[:, :, gsl])

                # ================= Output head + avg_attn =================
                hfin = h_a if L % 2 == 0 else h_b
                for n in range(NCH):
                    ps = psA.tile([1, CW], F32, tag="big")
                    for kt in range(DT):
                        nc.tensor.matmul(
                            ps[:], owt[:, kt : kt + 1],
                            hfin[:, kt, bass.ds(n * CW, CW)],
                            start=(kt == 0), stop=(kt == DT - 1),
                        )
                    pl = linp.tile([1, CW], F32, tag="predl")
                    nc.scalar.copy(pl[:], ps[:])
                    if out_b_val != 0.0:
                        nc.vector.tensor_scalar_add(pl[:], pl[:], out_b_val)
                    nc.sync.dma_start(d_pred[:, bass.ds(n * CW, CW)], pl[:])
                for r in range(NROW):
                    fin = midp.tile([P, 2, V], F32, tag="avfin")
                    nc.scalar.activation(
                        fin[:], acc[:, 2 * r : 2 * r + 2, :], AF.Copy,
                        scale=1.0 / (L * H),
                    )
                    nc.sync.dma_start(
                        d_avg[bass.ds(r * V, V), :].rearrange(
                            "(qt p) k -> p qt k", p=P
                        ),
                        fin[:],
                    )
    return nc


def kernel(x, mask, value, params):
    global LAST_RESULT
    x = np.asarray(x, np.float32)
    mask = np.asarray(mask, np.float32)
    value = np.asarray(value, np.float32)
    pins, out_b_val = _prep_params(params)
    nc = _build(out_b_val)
    if not nc.is_finalized():
        nc.finalize()
    in_maps = []
    for c in range(NCORES):
        rows = slice(c * NROW, (c + 1) * NROW)
        embal = np.concatenate([
            np.broadcast_to(x[rows].reshape(1, TOK), (P, TOK)),
            np.broadcast_to(value[rows].reshape(1, TOK), (P, TOK)),
            np.broadcast_to(mask[rows].reshape(1, TOK), (P, TOK)),
            pins["_embtail"],
        ], axis=1).astype(np.float32)
        im = {"embal": np.ascontiguousarray(embal)}
        im.update({k: v for k, v in pins.items() if k != "_embtail"})
        in_maps.append(im)
    import os
    res = run_bass_kernel_spmd(
        nc, in_maps, core_ids=list(range(NCORES)),
        trace=bool(os.environ.get("BASS_TRACE")),
    )
    LAST_RESULT = res
    pred = np.concatenate(
        [res.results[c]["pred"].reshape(NROW, V) for c in range(NCORES)], 0
    )
    avg = np.concatenate(
        [res.results[c]["avg"].reshape(NROW, V, V) for c in range(NCORES)], 0
    )
    return pred, avg


# revision 20
# speedup vs baseline: 1.0882x; 1.0882x over previous
# Trainium2 Bass kernel for nn_CausalTransformer (B=64, V=256, D=512, DFF=2048, L=8, H=8).
# Data-parallel over batch: 8 rows per core x 8 cores, params replicated (no collectives).
# Activations live feature-major in SBUF ([feature-partition, token-free]); matmuls in
# bf16 with f32 PSUM accumulation; weights are pre-transposed + bf16-cast on host.
import numpy as np
import ml_dtypes

import concourse.bass as bass
import concourse.mybir as mybir
import concourse.tile as tile
from concourse import bacc
from concourse.bass_utils import run_bass_kernel_spmd
from concourse.masks import make_identity

P = 128
B, V, D, DFF, L, H = 64, 256, 512, 2048, 8, 8
HD = D // H            # 64
NCORES = 8
NROW = B // NCORES     # 8 batch rows per core
TOK = NROW * V         # 2048 tokens per core
DT = D // P            # 4 feature tiles
FT = DFF // P          # 16 ffn tiles
CW = 512               # token chunk width
NCH = TOK // CW        # 4 chunks
EPS = 1e-5
F32 = mybir.dt.float32
BF16 = mybir.dt.bfloat16
AF = mybir.ActivationFunctionType
ALU = mybir.AluOpType

LAST_RESULT = None  # BassKernelResults of the most recent run (for profiling)


def _bf(a):
    return np.ascontiguousarray(a.astype(ml_dtypes.bfloat16))


def _f32(a):
    return np.ascontiguousarray(a.astype(np.float32))


def _fm_tiles(w):
    # [Dout, Din] weight -> lhsT tiles [P, Din//P, Dout] (partition = Din within tile)
    dout, din = w.shape
    return np.ascontiguousarray(w.T.reshape(din // P, P, dout).transpose(1, 0, 2))


def _pcol(v):
    # [N] vector -> [P, N//P] column-per-tile layout (partition-major within tile)
    n = v.shape[0]
    return np.ascontiguousarray(v.reshape(n // P, P).T)


def _prep_params(params):
    p = {k: np.asarray(v) for k, v in params.items()}
    ins = {}
    ins["wqkvT"] = _bf(np.stack([_fm_tiles(p["Wqkv"][l]) for l in range(L)]))
    ins["bqkv"] = _f32(np.stack([_pcol(p["bqkv"][l]) for l in range(L)]))
    ins["woT"] = _bf(np.stack([_fm_tiles(p["Wo"][l]) for l in range(L)]))
    ins["w1T"] = _bf(np.stack([_fm_tiles(p["W1"][l]) for l in range(L)]))
    ins["b1"] = _f32(np.stack([_pcol(p["b1"][l]) for l in range(L)]))
    ins["w2T"] = _bf(np.stack([_fm_tiles(p["W2"][l]) for l in range(L)]))
    ins["ln1g"] = _f32(np.stack([_pcol(p["ln1_g"][l]) for l in range(L)]))
    ins["ln1b"] = _f32(np.stack([_pcol(p["ln1_b"][l]) for l in range(L)]))
    ins["ln2g"] = _f32(np.stack([_pcol(p["ln2_g"][l]) for l in range(L)]))
    ins["ln2b"] = _f32(np.stack([_pcol(p["ln2_b"][l]) for l in range(L)]))
    for nm in ("fe", "ve"):
        ins[nm + "w2T"] = _bf(_fm_tiles(p[nm + "_w2"]))
    sm = np.concatenate([
        _pcol(p["fe_w1"][:, 0]), _pcol(p["fe_b1"]),
        _pcol(p["ve_w1"][:, 0]), _pcol(p["ve_b1"]),
        _pcol(p["fe_g"]), _pcol(p["fe_bn"]), _pcol(p["ve_g"]), _pcol(p["ve_bn"]),
        _pcol(p["fe_b2"]), _pcol(p["ve_b2"]),
        _pcol(p["me_w"][:, 0]), _pcol(p["me_b"]),
    ], axis=1)                                   # [P, 64]
    vet = p["var_emb"].T.reshape(DT, P, V).transpose(1, 0, 2).reshape(P, DT * V)
    ins["_embtail"] = _f32(np.concatenate([sm, vet], axis=1))   # [P, 64+1024]
    ins["owT"] = _bf(_pcol(p["out_w"][0]))
    out_b = float(np.asarray(p["out_b"]).reshape(-1)[0])
    return ins, out_b


def _build(out_b_val):
    nc = bacc.Bacc()
    # ---- DRAM I/O ----
    d_emb = nc.dram_tensor("embal", [P, 3 * TOK + 64 + DT * V], F32,
                           kind="ExternalInput")
    shp = {
        "wqkvT": ([L, P, DT, 3 * D], BF16), "bqkv": ([L, P, 12], F32),
        "woT": ([L, P, DT, D], BF16),
        "w1T": ([L, P, DT, DFF], BF16), "b1": ([L, P, FT], F32),
        "w2T": ([L, P, FT, D], BF16),
        "ln1g": ([L, P, DT], F32), "ln1b": ([L, P, DT], F32),
        "ln2g": ([L, P, DT], F32), "ln2b": ([L, P, DT], F32),
        "few2T": ([P, 8, D], BF16),
        "vew2T": ([P, 8, D], BF16),
        "owT": ([P, DT], BF16),
    }
    dps = {k: nc.dram_tensor(k, s, dt, kind="ExternalInput") for k, (s, dt) in shp.items()}
    d_pred = nc.dram_tensor("pred", [1, TOK], F32, kind="ExternalOutput")
    d_avg = nc.dram_tensor("avg", [TOK, V], F32, kind="ExternalOutput")

    with tile.TileContext(nc) as tc:
        with (
            tc.tile_pool(name="state", bufs=1) as st,
            tc.tile_pool(name="lnp", bufs=2) as lnp,
            tc.tile_pool(name="linp", bufs=2) as linp,
            tc.tile_pool(name="smallp", bufs=4) as smallp,
            tc.tile_pool(name="psA", bufs=3, space="PSUM") as psA,
            tc.tile_pool(name="psS", bufs=2, space="PSUM") as psS,
            tc.tile_pool(name="psV", bufs=1, space="PSUM") as psV,
            tc.tile_pool(name="psT", bufs=1, space="PSUM") as psT,
        ):
            # ---- persistent state ----
            h_a = st.tile([P, DT, TOK], BF16, tag="h_a")
            h_b = st.tile([P, DT, TOK], BF16, tag="h_b")
            acc = st.tile([P, 2 * NROW, V], F32, tag="acc")  # [q%128, 2*r+qt, k]
            ident = st.tile([P, P], BF16, tag="ident")
            ones_bf = st.tile([P, 1], BF16, tag="ones_bf")
            ones_f2 = st.tile([P, 1], F32, tag="ones_f2")
            erow = st.tile([P, P], F32, tag="erow")
            owt = st.tile([P, DT], BF16, tag="owt")

            make_identity(nc, ident[:])
            nc.vector.memset(ones_bf[:], 1.0)
            nc.vector.memset(ones_f2[:], 1.0)
            nc.vector.memset(erow[:], 0.0)
            nc.vector.memset(erow[0:1, :], 1.0)
            nc.vector.memset(acc[:], 0.0)
            nc.sync.dma_start(owt[:], dps["owT"][:])

            def ln_chunk(src, g_ap, b_ap, out_ap, add_into=None):
                # LayerNorm over D of feature-major chunk src [P, DT, CW] (bf16).
                sq = lnp.tile([P, DT, CW], BF16, tag="lnsq")
                nc.scalar.activation(sq[:], src, AF.Square)
                s1 = psA.tile([1, CW], F32, tag="big")
                s2 = psA.tile([1, CW], F32, tag="big")
                for kt in range(DT):
                    nc.tensor.matmul(s1[:], ones_f2[:], src[:, kt, :],
                                     start=(kt == 0), stop=(kt == DT - 1))
                for kt in range(DT):
                    nc.tensor.matmul(s2[:], ones_bf[:], sq[:, kt, :],
                                     start=(kt == 0), stop=(kt == DT - 1))
                slin = linp.tile([1, 2, CW], F32, tag="lin")
                nc.scalar.copy(slin[:, 0, :], s1[:])
                nc.scalar.copy(slin[:, 1, :], s2[:])
                sst = smallp.tile([32, 2, 16], F32, tag="sst")
                for j in range(2):
                    nc.sync.dma_start(
                        sst[:, j : j + 1, :],
                        slin[:, j, :].rearrange("o (p f) -> o p f", p=32),
                    )
                mean = smallp.tile([32, 16], F32, tag="lnm")
                e2 = smallp.tile([32, 16], F32, tag="lne2")
                var = smallp.tile([32, 16], F32, tag="lnvar")
                inv = smallp.tile([32, 16], F32, tag="lninv")
                minv = smallp.tile([32, 16], F32, tag="lnminv")
                nc.vector.tensor_scalar_mul(mean[:], sst[:, 0, :], 1.0 / D)
                nc.vector.tensor_scalar_mul(e2[:], sst[:, 1, :], 1.0 / D)
                nc.vector.tensor_tensor(var[:], mean[:], mean[:], ALU.mult)
                nc.vector.tensor_tensor(var[:], e2[:], var[:], ALU.subtract)
                nc.vector.tensor_scalar_add(var[:], var[:], EPS)
                nc.scalar.activation(var[:], var[:], AF.Sqrt)
                nc.vector.reciprocal(inv[:], var[:])
                nc.vector.tensor_tensor(minv[:], mean[:], inv[:], ALU.mult)
                vecs = linp.tile([P, 2, CW], F32, tag="vecs")
                nc.vector.memset(vecs[:], 0.0)
                nc.sync.dma_start(
                    vecs[0:1, 0, :].rearrange("o (p f) -> o p f", p=32), inv[:, None, :]
                )
                nc.sync.dma_start(
                    vecs[0:1, 1, :].rearrange("o (p f) -> o p f", p=32), minv[:, None, :]
                )
                invb = psA.tile([P, CW], F32, tag="big")
                minvb = psA.tile([P, CW], F32, tag="big")
                nc.tensor.matmul(invb[:], erow[:], vecs[:, 0, :])
                nc.tensor.matmul(minvb[:], erow[:], vecs[:, 1, :])
                for kt in range(DT):
                    t = lnp.tile([P, CW], F32, tag="lnt")
                    nc.vector.tensor_tensor(t[:], src[:, kt, :], invb[:], ALU.mult)
                    nc.vector.tensor_tensor(t[:], t[:], minvb[:], ALU.subtract)
                    if add_into is None:
                        nc.vector.tensor_scalar(
                            out_ap[:, kt, :], t[:], g_ap[:, kt : kt + 1],
                            b_ap[:, kt : kt + 1], ALU.mult, ALU.add,
                        )
                    else:
                        nc.vector.tensor_scalar(
                            t[:], t[:], g_ap[:, kt : kt + 1],
                            b_ap[:, kt : kt + 1], ALU.mult, ALU.add,
                        )
                        nc.vector.tensor_tensor(
                            add_into[:, kt, :], add_into[:, kt, :], t[:], ALU.add
                        )

            # ================= Embeddings (phase-scoped pools) =================
            with (
                tc.tile_pool(name="embw", bufs=1) as ew,
                tc.tile_pool(name="embp", bufs=2) as ep,
            ):
                embal = ew.tile([P, 3 * TOK + 64 + DT * V], F32, tag="embal")
                fw2 = ew.tile([P, 8, D], BF16, tag="few2T")
                vw2 = ew.tile([P, 8, D], BF16, tag="vew2T")
                nc.sync.dma_start(embal[:], d_emb[:])
                nc.sync.dma_start(fw2[:], dps["few2T"][:])
                nc.sync.dma_start(vw2[:], dps["vew2T"][:])
                xbt = embal[:, 0:TOK]
                vbt = embal[:, TOK : 2 * TOK]
                mbt = embal[:, 2 * TOK : 3 * TOK]
                s0 = 3 * TOK
                emb = {
                    "few1": embal[:, s0 : s0 + 8], "feb1": embal[:, s0 + 8 : s0 + 16],
                    "vew1": embal[:, s0 + 16 : s0 + 24],
                    "veb1": embal[:, s0 + 24 : s0 + 32],
                    "feg": embal[:, s0 + 32 : s0 + 36],
                    "febn": embal[:, s0 + 36 : s0 + 40],
                    "veg": embal[:, s0 + 40 : s0 + 44],
                    "vebn": embal[:, s0 + 44 : s0 + 48],
                    "feb2": embal[:, s0 + 48 : s0 + 52],
                    "veb2": embal[:, s0 + 52 : s0 + 56],
                    "mew": embal[:, s0 + 56 : s0 + 60],
                    "meb": embal[:, s0 + 60 : s0 + 64],
                    "few2T": fw2, "vew2T": vw2,
                    "varembT": embal[:, s0 + 64 :].rearrange(
                        "p (kt v) -> p kt v", v=V),
                }
                for g in range(NCH):
                    gsl = bass.ds(g * CW, CW)
                    xb = xbt[:, gsl]
                    vb = vbt[:, gsl]
                    mb = mbt[:, gsl]
                    for nm, srcb in (("fe", xb), ("ve", vb)):
                        h1 = ep.tile([P, 8, CW], BF16, tag="embh1")
                        for m in range(8):
                            nc.scalar.activation(
                                h1[:, m, :], srcb, AF.Gelu,
                                scale=emb[nm + "w1"][:, m : m + 1],
                                bias=emb[nm + "b1"][:, m : m + 1],
                            )
                        esb = ep.tile([P, DT, CW], F32, tag="esb")
                        for m in range(DT):
                            ps = psA.tile([P, CW], F32, tag="big")
                            for kt in range(8):
                                nc.tensor.matmul(
                                    ps[:], emb[nm + "w2T"][:, kt, bass.ds(m * P, P)],
                                    h1[:, kt, :], start=(kt == 0), stop=(kt == 7),
                                )
                            nc.vector.tensor_scalar_add(
                                esb[:, m, :], ps[:], emb[nm + "b2"][:, m : m + 1]
                            )
                        if nm == "fe":
                            ln_chunk(esb[:], emb["feg"], emb["febn"], h_a[:, :, gsl])
                        else:
                            ln_chunk(esb[:], emb["veg"], emb["vebn"], None,
                                     add_into=h_a[:, :, gsl])
                    for kt in range(DT):
                        met = ep.tile([P, CW], F32, tag="met")
                        nc.vector.tensor_scalar(
                            met[:], mb, emb["mew"][:, kt : kt + 1],
                            emb["meb"][:, kt : kt + 1], ALU.mult, ALU.add,
                        )
                        nc.vector.tensor_tensor(
                            h_a[:, kt, gsl], h_a[:, kt, gsl], met[:], ALU.add
                        )
                        nc.vector.tensor_tensor(
                            h_a[:, kt, gsl].rearrange("p (r k) -> p r k", k=V),
                            h_a[:, kt, gsl].rearrange("p (r k) -> p r k", k=V),
                            emb["varembT"][:, kt, None, :].to_broadcast((P, CW // V, V)),
                            ALU.add,
                        )

            # ================= Transformer layers =================
            with (
                tc.tile_pool(name="wp", bufs=1) as wp,
                tc.tile_pool(name="qkvp", bufs=1) as qkvp,
                tc.tile_pool(name="attp", bufs=2) as attp,
                tc.tile_pool(name="att1", bufs=1) as att1,
                tc.tile_pool(name="midp", bufs=2) as midp,
                tc.tile_pool(name="mid1", bufs=1) as mid1,
            ):
                for l in range(L):
                    hin = h_a if l % 2 == 0 else h_b
                    hout = h_b if l % 2 == 0 else h_a
                    wqkv = wp.tile([P, DT, 3 * D], BF16, tag="wqkv")
                    bqkv = wp.tile([P, 12], F32, tag="bqkv")
                    wo = wp.tile([P, DT, D], BF16, tag="wo")
                    w1 = wp.tile([P, DT, DFF], BF16, tag="w1")
                    b1 = wp.tile([P, FT], F32, tag="b1")
                    w2 = wp.tile([P, FT, D], BF16, tag="w2")
                    lng = wp.tile([P, 4, DT], F32, tag="lng")
                    nc.sync.dma_start(wqkv[:], dps["wqkvT"][l])
                    nc.sync.dma_start(bqkv[:], dps["bqkv"][l])
                    nc.sync.dma_start(wo[:], dps["woT"][l])
                    nc.sync.dma_start(w1[:], dps["w1T"][l])
                    nc.sync.dma_start(b1[:], dps["b1"][l])
                    nc.sync.dma_start(w2[:], dps["w2T"][l])
                    for j, k in enumerate(("ln1g", "ln1b", "ln2g", "ln2b")):
                        nc.sync.dma_start(lng[:, j, :], dps[k][l])

                    for g in range(NCH):
                        gsl = bass.ds(g * CW, CW)
                        # ---- QKV projection ----
                        qkv = qkvp.tile([P, 12, CW], BF16, tag="qkv")
                        for m in range(12):
                            ps = psA.tile([P, CW], F32, tag="big")
                            for kt in range(DT):
                                nc.tensor.matmul(
                                    ps[:], wqkv[:, kt, bass.ds(m * P, P)],
                                    hin[:, kt, gsl],
                                    start=(kt == 0), stop=(kt == DT - 1),
                                )
                            nc.scalar.activation(
                                qkv[:, m, :], ps[:], AF.Identity,
                                bias=bqkv[:, m : m + 1],
                            )
                        # ---- attention (2 rows in this chunk) ----
                        attn = attp.tile([P, DT, CW], BF16, tag="attn")
                        for rr in range(2):
                            r = 2 * g + rr
                            rsl = bass.ds(rr * V, V)
                            psb = att1.tile([P, H, 2, V], BF16, tag="psb")
                            sums = smallp.tile([P, 2, H], F32, tag="sums")
                            for hh in range(H):
                                hp, hq = (hh % 2) * 64, hh // 2
                                for qt in range(2):
                                    scps = psS.tile([P, V], F32, tag="sc")
                                    nc.tensor.matmul(
                                        scps[:],
                                        qkv[hp : hp + 64, hq,
                                            bass.ds(rr * V + qt * P, P)],
                                        qkv[hp : hp + 64, 4 + hq, rsl],
                                    )
                                    nc.scalar.activation(
                                        psb[:, hh, qt, :], scps[:], AF.Exp,
                                        scale=0.125,
                                        accum_out=sums[:, qt, hh : hh + 1],
                                    )
                            rinv = smallp.tile([P, 2, H], F32, tag="rinv")
                            nc.vector.reciprocal(rinv[:], sums[:])
                            for hh in range(H):
                                hp, hq = (hh % 2) * 64, hh // 2
                                ptT = attp.tile([P, 2, V], BF16, tag="ptT")
                                for qt in range(2):
                                    pn = attp.tile([P, V], BF16, tag="pn")
                                    nc.vector.tensor_scalar_mul(
                                        pn[:], psb[:, hh, qt, :],
                                        rinv[:, qt, hh : hh + 1],
                                    )
                                    nc.vector.tensor_tensor(
                                        acc[:, 2 * r + qt, :], acc[:, 2 * r + qt, :],
                                        pn[:], ALU.add,
                                    )
                                    tp = psT.tile([P, 2, P], BF16, tag="tr")
                                    for kt in range(2):
                                        nc.tensor.transpose(
                                            tp[:, kt, :], pn[:, bass.ds(kt * P, P)],
                                            ident[:],
                                        )
                                    nc.vector.tensor_copy(
                                        ptT[:, :, bass.ds(qt * P, P)], tp[:]
                                    )
                                vtok = attp.tile([P, 2, HD], BF16, tag="vtok")
                                tv = psT.tile([P, 2, HD], BF16, tag="tv")
                                for kt in range(2):
                                    nc.tensor.transpose(
                                        tv[:, kt, :],
                                        qkv[hp : hp + 64, 8 + hq,
                                            bass.ds(rr * V + kt * P, P)],
                                        ident[hp : hp + 64, bass.ds(hp, 64)],
                                    )
                                nc.vector.tensor_copy(vtok[:], tv[:])
                                avps = psV.tile([64, V], F32, tag="av")
                                for qt in range(2):
                                    for kt in range(2):
                                        nc.tensor.matmul(
                                            avps[:, bass.ds(qt * P, P)],
                                            vtok[:, kt, :],
                                            ptT[:, kt, bass.ds(qt * P, P)],
                                            start=(kt == 0), stop=(kt == 1),
                                        )
                                nc.scalar.copy(attn[hp : hp + 64, hq, rsl], avps[:])
                        # ---- Wo + residual ----
                        hmid = mid1.tile([P, DT, CW], F32, tag="hmid")
                        for m in range(DT):
                            ps = psA.tile([P, CW], F32, tag="big")
                            for kt in range(DT):
                                nc.tensor.matmul(
                                    ps[:], wo[:, kt, bass.ds(m * P, P)],
                                    attn[:, kt, :],
                                    start=(kt == 0), stop=(kt == DT - 1),
                                )
                            nc.vector.tensor_tensor(
                                hmid[:, m, :], ps[:], hin[:, m, gsl], ALU.add
                            )
                        # ---- LN1 ----
                        hln = midp.tile([P, DT, CW], BF16, tag="hln")
                        ln_chunk(hmid[:], lng[:, 0, :], lng[:, 1, :], hln)
                        # ---- FFN ----
                        mid = mid1.tile([P, FT, CW], BF16, tag="mid")
                        for m in range(FT):
                            ps = psA.tile([P, CW], F32, tag="big")
                            for kt in range(DT):
                                nc.tensor.matmul(
                                    ps[:], w1[:, kt, bass.ds(m * P, P)],
                                    hln[:, kt, :],
                                    start=(kt == 0), stop=(kt == DT - 1),
                                )
                            nc.scalar.activation(
                                mid[:, m, :], ps[:], AF.Relu, bias=b1[:, m : m + 1]
                            )
                        h2m = mid1.tile([P, DT, CW], F32, tag="h2m")
                        for m in range(DT):
                            ps = psA.tile([P, CW], F32, tag="big")
                            for kt in range(FT):
                                nc.tensor.matmul(
                                    ps[:], w2[:, kt, bass.ds(m * P, P)],
                                    mid[:, kt, :],
                                    start=(kt == 0), stop=(kt == FT - 1),
                                )
                            nc.vector.tensor_tensor(
                                h2m[:, m, :], ps[:], hln[:, m, :], ALU.add
                            )
                        # ---- LN2 ----
                        ln_chunk(h2m[:], lng[:, 2, :], lng[:, 3, :], hout[:, :, gsl])

                # ================= Output head + avg_attn =================
                hfin = h_a if L % 2 == 0 else h_b
                for n in range(NCH):
                    ps = psA.tile([1, CW], F32, tag="big")
                    for kt in range(DT):
                        nc.tensor.matmul(
                            ps[:], owt[:, kt : kt + 1],
                            hfin[:, kt, bass.ds(n * CW, CW)],
                            start=(kt == 0), stop=(kt == DT - 1),
                        )
                    pl = linp.tile([1, CW], F32, tag="predl")
                    nc.scalar.copy(pl[:], ps[:])
                    if out_b_val != 0.0:
                        nc.vector.tensor_scalar_add(pl[:], pl[:], out_b_val)
                    nc.sync.dma_start(d_pred[:, bass.ds(n * CW, CW)], pl[:])
                for r in range(NROW):
                    fin = midp.tile([P, 2, V], F32, tag="avfin")
                    nc.scalar.activation(
                        fin[:], acc[:, 2 * r : 2 * r + 2, :], AF.Copy,
                        scale=1.0 / (L * H),
                    )
                    nc.sync.dma_start(
                        d_avg[bass.ds(r * V, V), :].rearrange(
                            "(qt p) k -> p qt k", p=P
                        ),
                        fin[:],
                    )
    return nc


def kernel(x, mask, value, params):
    global LAST_RESULT
    x = np.asarray(x, np.float32)
    mask = np.asarray(mask, np.float32)
    value = np.asarray(value, np.float32)
    pins, out_b_val = _prep_params(params)
    nc = _build(out_b_val)
    if not nc.is_finalized():
        nc.finalize()
    in_maps = []
    for c in range(NCORES):
        rows = slice(c * NROW, (c + 1) * NROW)
        embal = np.concatenate([
            np.broadcast_to(x[rows].reshape(1, TOK), (P, TOK)),
            np.broadcast_to(value[rows].reshape(1, TOK), (P, TOK)),
            np.broadcast_to(mask[rows].reshape(1, TOK), (P, TOK)),
            pins["_embtail"],
        ], axis=1).astype(np.float32)
        im = {"embal": np.ascontiguousarray(embal)}
        im.update({k: v for k, v in pins.items() if k != "_embtail"})
        in_maps.append(im)
    import os
    res = run_bass_kernel_spmd(
        nc, in_maps, core_ids=list(range(NCORES)),
        trace=bool(os.environ.get("BASS_TRACE")),
    )
    LAST_RESULT = res
    pred = np.concatenate(
        [res.results[c]["pred"].reshape(NROW, V) for c in range(NCORES)], 0
    )
    avg = np.concatenate(
        [res.results[c]["avg"].reshape(NROW, V, V) for c in range(NCORES)], 0
    )
    return pred, avg
